# revision 19
# baseline (speedup 1.0000x reference)
"""AttentionBlock (GroupNorm + 4-head self-attention + proj + residual) on
8 TRN2 NeuronCores.

Sharding: core i handles batch i//2, query-half i%2 (2048 of 4096 pixels),
all 4 heads. Zero collectives: each core computes GroupNorm + QKV locally
(keys/values need all pixels anyway), attention for its query slice, and the
output projection + residual for its slice. Host reassembles disjoint slices.

The host rotates each core's pixel axis so its own queries are always
columns 0:2048 — the compiled graph is identical on all cores (SPMD), and
softmax is permutation-invariant along the key axis.
"""
import numpy as np

B, C = 4, 256
N = 64 * 64          # pixels
NH, HD = 4, 64       # heads, head dim
NG = 8               # groupnorm groups
EPS = 1e-5
NLOC = N // 2        # queries per core
GSIZE = (C // NG) * N  # elements per group = 131072

_CACHE: dict = {}


def _build_graph():
    import concourse.bass as bass
    import concourse.tile as tile
    from concourse import bacc, mybir

    F32 = mybir.dt.float32
    F32R = mybir.dt.float32r
    AX = mybir.AxisListType.X
    OP = mybir.AluOpType
    EXP = mybir.ActivationFunctionType.Exp

    nc = bacc.Bacc()

    # ---- DRAM I/O ----
    x_d = nc.dram_tensor("x", [C, N], F32, kind="ExternalInput")
    wqt_d = nc.dram_tensor("wqt", [C, C], F32R, kind="ExternalInput")
    wkt_d = nc.dram_tensor("wkt", [C, C], F32R, kind="ExternalInput")
    wvt_d = nc.dram_tensor("wvt", [C, C], F32R, kind="ExternalInput")
    pjt_d = nc.dram_tensor("pjt", [C, C], F32R, kind="ExternalInput")
    qb_d = nc.dram_tensor("qb", [C, 1], F32, kind="ExternalInput")
    kb_d = nc.dram_tensor("kb", [C, 1], F32, kind="ExternalInput")
    pb_d = nc.dram_tensor("pb", [C, 1], F32, kind="ExternalInput")
    gnw_d = nc.dram_tensor("gnw", [C, 1], F32, kind="ExternalInput")
    gnb_d = nc.dram_tensor("gnb", [C, 1], F32, kind="ExternalInput")
    gt_d = nc.dram_tensor("gt", [128, 128], F32, kind="ExternalInput")
    on_d = nc.dram_tensor("on", [128, 128], F32, kind="ExternalInput")
    out_d = nc.dram_tensor("out", [C, NLOC], F32, kind="ExternalOutput")

    with tile.TileContext(nc) as tc:
        with (
            tc.tile_pool(name="big", bufs=1) as big,
            tc.tile_pool(name="small", bufs=1) as small,
        ):
            xnp_cm = tc.tile_pool(name="xnp", bufs=1)
            xnp = xnp_cm.__enter__()
            psA_cm = tc.tile_pool(name="psA", bufs=2, space="PSUM")
            psA = psA_cm.__enter__()
            # ---- load x + params ----
            xs = []
            for t in range(2):
                xt = big.tile([128, N], F32, tag=f"x{t}")
                nc.sync.dma_start(out=xt, in_=x_d[t * 128:(t + 1) * 128, :])
                xs.append(xt)
            gt = small.tile([128, 128], F32, tag="gt")
            nc.sync.dma_start(out=gt, in_=gt_d[:, :])
            wq, wk, wv, pjt = [], [], [], []
            for t in range(2):
                for lst, src in ((wq, wqt_d), (wk, wkt_d), (wv, wvt_d)):
                    w = small.tile([128, C], F32R, tag=f"w{len(lst)}{src.name}")
                    nc.sync.dma_start(out=w, in_=src[t * 128:(t + 1) * 128, :])
                    lst.append(w)
            for h in range(NH):
                w = small.tile([64, C], F32R, tag=f"pj{h}")
                nc.sync.dma_start(out=w, in_=pjt_d[h * 64:(h + 1) * 64, :])
                pjt.append(w)
            par = {}
            for name, src in (("qb", qb_d), ("kb", kb_d), ("pb", pb_d),
                              ("gnw", gnw_d), ("gnb", gnb_d)):
                for t in range(2):
                    p = small.tile([128, 1], F32, tag=f"{name}{t}")
                    nc.sync.dma_start(out=p, in_=src[t * 128:(t + 1) * 128, :])
                    par[(name, t)] = p

            # ---- GroupNorm -> xn (f32r) ----
            xn = []
            for t in range(2):
                xnt = xnp.tile([128, N], F32R, tag=f"xn{t}")
                st2 = small.tile([128, 2], F32, tag=f"st2_{t}")
                # sum(x) and sum(x^2); xnt doubles as the x^2 scratch
                nc.vector.reduce_sum(out=st2[:, 0:1], in_=xs[t], axis=AX)
                nc.vector.tensor_mul(xnt, xs[t], xs[t])
                nc.vector.reduce_sum(out=st2[:, 1:2], in_=xnt, axis=AX)
                gps = psA.tile([128, 2], F32, tag="gps")
                nc.tensor.matmul(gps, gt, st2, start=True, stop=True)
                mean = small.tile([128, 1], F32, tag=f"mean{t}")
                ve = small.tile([128, 1], F32, tag=f"ve{t}")
                nc.vector.tensor_scalar_mul(mean, gps[:, 0:1], 1.0 / GSIZE)
                # (E[x^2] - mean^2) + eps
                msq = small.tile([128, 1], F32, tag=f"msq{t}")
                nc.vector.tensor_mul(msq, mean, mean)
                nc.vector.tensor_scalar(
                    out=ve, in0=gps[:, 1:2], scalar1=1.0 / GSIZE, scalar2=msq,
                    op0=OP.mult, op1=OP.subtract)
                nc.vector.tensor_scalar_add(ve, ve, EPS)
                rv = small.tile([128, 1], F32, tag=f"rv{t}")
                nc.vector.reciprocal(rv, ve)
                rstd = small.tile([128, 1], F32, tag=f"rstd{t}")
                nc.scalar.sqrt(rstd, rv)
                av = small.tile([128, 1], F32, tag=f"av{t}")
                bv = small.tile([128, 1], F32, tag=f"bv{t}")
                nc.vector.tensor_mul(av, rstd, par[("gnw", t)])
                # bv = gnb - mean*av
                nc.vector.tensor_mul(bv, mean, av)
                nc.vector.scalar_tensor_tensor(
                    out=bv, in0=bv, scalar=-1.0, in1=par[("gnb", t)],
                    op0=OP.mult, op1=OP.add)
                nc.vector.tensor_scalar(
                    out=xnt, in0=xs[t], scalar1=av, scalar2=bv,
                    op0=OP.mult, op1=OP.add)
                xn.append(xnt)

            # ---- QKV matmuls (f32r) ----
            # q: (C,2048) packed head-pairs; k: (C,4096); vT: per head (128,32,65)
            qsb, ksb = [], []
            for t in range(2):
                qt = big.tile([128, NLOC], F32R, tag=f"q{t}")
                kt = big.tile([128, N], F32R, tag=f"k{t}")
                for j in range(NLOC // 512):
                    qp = psA.tile([128, 512], F32, tag="qkv")
                    for cc in range(2):
                        nc.tensor.matmul(
                            qp, wq[cc][:, t * 128:(t + 1) * 128],
                            xn[cc][:, j * 512:(j + 1) * 512],
                            start=(cc == 0), stop=(cc == 1))
                    nc.vector.tensor_scalar_add(
                        qt[:, j * 512:(j + 1) * 512], qp, par[("qb", t)])
                for j in range(N // 512):
                    kp = psA.tile([128, 512], F32, tag="qkv")
                    for cc in range(2):
                        nc.tensor.matmul(
                            kp, wk[cc][:, t * 128:(t + 1) * 128],
                            xn[cc][:, j * 512:(j + 1) * 512],
                            start=(cc == 0), stop=(cc == 1))
                    nc.vector.tensor_scalar_add(
                        kt[:, j * 512:(j + 1) * 512], kp, par[("kb", t)])
                qsb.append(qt)
                ksb.append(kt)

            vt = big.tile([128, NH, N // 128, 65], F32R, tag="vt")
            # denominator ones-column (col 64 of each (h, mc) block)
            onsb = small.tile([128, 128], F32, tag="onsb")
            nc.sync.dma_start(out=onsb, in_=on_d[:, :])
            nc.vector.tensor_copy(
                vt[:, :, :, 64:65],
                onsb.rearrange("p (a b) -> p a b", a=NH))
            for mc in range(N // 128):
                vp = psA.tile([128, C], F32, tag="vtp")
                for cc in range(2):
                    nc.tensor.matmul(
                        vp, xn[cc][:, mc * 128:(mc + 1) * 128], wv[cc],
                        start=(cc == 0), stop=(cc == 1))
                nc.vector.tensor_copy(
                    vt[:, :, mc, 0:64],
                    vp.rearrange("p (h d) -> p h d", h=NH))

            psA_cm.__exit__(None, None, None)
            xnp_cm.__exit__(None, None, None)
            psB_cm = tc.tile_pool(name="psB", bufs=2, space="PSUM")
            psB = psB_cm.__enter__()
            epool_cm = tc.tile_pool(name="epool", bufs=3)
            epool = epool_cm.__enter__()
            bcpool_cm = tc.tile_pool(name="bcpool", bufs=2)
            bcpool = bcpool_cm.__enter__()
            drp_cm = tc.tile_pool(name="drp", bufs=2, space="DRAM")
            drp = drp_cm.__enter__()
            attp_cm = tc.tile_pool(name="att", bufs=1)
            attp = attp_cm.__enter__()
            ypool_cm = tc.tile_pool(name="ypool", bufs=2)
            ypool = ypool_cm.__enter__()

            # ---- attention per head ----
            att = []
            for h in range(NH):
                t, hh = h // 2, h % 2
                ah = attp.tile([64, NLOC], F32R, tag=f"att{h}")
                att.append(ah)
                for Hw in range(2):
                    q_sl = qsb[t][hh * 64:(hh + 1) * 64,
                                  Hw * 1024:(Hw + 1) * 1024]
                    pv = psB.tile([65, 1024], F32, tag="pv")
                    for kb in range(N // 128):
                        st = psB.tile([128, 1024], F32, tag="st")
                        k_sl = ksb[t][hh * 64:(hh + 1) * 64,
                                      kb * 128:(kb + 1) * 128]
                        for j in range(2):
                            nc.tensor.matmul(
                                st[:, j * 512:(j + 1) * 512],
                                k_sl, q_sl[:, j * 512:(j + 1) * 512],
                                start=True, stop=True)
                        e = epool.tile([128, 1024], F32R, tag="e")
                        nc.scalar.activation(e, st, EXP)
                        for j in range(2):
                            nc.tensor.matmul(
                                pv[:, j * 512:(j + 1) * 512],
                                vt[:, h, kb, :], e[:, j * 512:(j + 1) * 512],
                                start=(kb == 0), stop=(kb == N // 128 - 1))
                    recip = small.tile([1, 1024], F32, tag="recip")
                    nc.vector.reciprocal(recip, pv[64:65, :])
                    # partition-broadcast recip to 64 rows: SBUF -> DRAM ->
                    # SBUF with a step-0 partition AP (legal for DRAM source)
                    rd = drp.tile([1, 1024], F32, tag="rd")
                    nc.sync.dma_start(out=rd, in_=recip)
                    bc = bcpool.tile([64, 1024], F32, tag="bc")
                    nc.sync.dma_start(
                        out=bc,
                        in_=bass.AP(tensor=rd.tensor, offset=rd.offset,
                                    ap=[[0, 64], [1, 1024]]))
                    nc.vector.tensor_mul(
                        ah[:, Hw * 1024:(Hw + 1) * 1024], pv[0:64, :], bc)

            # ---- proj + bias + residual ----
            for Hw in range(2):
                for o in range(2):
                    yp = psB.tile([128, 1024], F32, tag="pv")
                    for j in range(2):
                        for h in range(NH):
                            nc.tensor.matmul(
                                yp[:, j * 512:(j + 1) * 512],
                                pjt[h][:, o * 128:(o + 1) * 128],
                                att[h][:, Hw * 1024 + j * 512:
                                       Hw * 1024 + (j + 1) * 512],
                                start=(h == 0), stop=(h == NH - 1))
                    ysb = ypool.tile([128, 1024], F32, tag="y")
                    # y = (yp + pb) + x_residual
                    nc.vector.scalar_tensor_tensor(
                        out=ysb, in0=yp, scalar=par[("pb", o)],
                        in1=xs[o][:, Hw * 1024:(Hw + 1) * 1024],
                        op0=OP.add, op1=OP.add)
                    nc.sync.dma_start(
                        out=out_d[o * 128:(o + 1) * 128,
                                  Hw * 1024:(Hw + 1) * 1024],
                        in_=ysb)
            ypool_cm.__exit__(None, None, None)
            attp_cm.__exit__(None, None, None)
            drp_cm.__exit__(None, None, None)
            bcpool_cm.__exit__(None, None, None)
            epool_cm.__exit__(None, None, None)
            psB_cm.__exit__(None, None, None)
    nc.compile()
    return nc


def kernel(**inputs):
    import concourse.bass_utils as bass_utils

    x = np.asarray(inputs["x"], np.float32)
    gn_w = np.asarray(inputs["gn_w"], np.float32)
    gn_b = np.asarray(inputs["gn_b"], np.float32)
    qkv_w = np.asarray(inputs["qkv_w"], np.float32)
    qkv_b = np.asarray(inputs["qkv_b"], np.float32)
    proj_w = np.asarray(inputs["proj_w"], np.float32)
    proj_b = np.asarray(inputs["proj_b"], np.float32)

    scale = HD ** -0.5
    wqt = np.ascontiguousarray((qkv_w[0:C] * scale).T)      # (C, C)
    wkt = np.ascontiguousarray(qkv_w[C:2 * C].T)
    wvt = np.ascontiguousarray(qkv_w[2 * C:3 * C].T)
    pjt = np.ascontiguousarray(proj_w.T)
    qb = (qkv_b[0:C] * scale).reshape(C, 1)
    kb = qkv_b[C:2 * C].reshape(C, 1)
    # v bias folds into the projection bias: proj(out + vb) = proj(out) + W@vb
    pb = (proj_b + proj_w @ qkv_b[2 * C:3 * C]).reshape(C, 1)
    gt = np.zeros((128, 128), np.float32)
    for g in range(4):
        gt[g * 32:(g + 1) * 32, g * 32:(g + 1) * 32] = 1.0

    if "nc" not in _CACHE:
        _CACHE["nc"] = _build_graph()
    nc = _CACHE["nc"]

    xb = x.reshape(B, C, N)
    in_maps = []
    for core in range(8):
        bb, half = core // 2, core % 2
        xl = np.concatenate(
            [xb[bb][:, half * NLOC:(half + 1) * NLOC],
             xb[bb][:, (1 - half) * NLOC:(2 - half) * NLOC]], axis=1)
        in_maps.append({
            "x": np.ascontiguousarray(xl), "wqt": wqt, "wkt": wkt,
            "wvt": wvt, "pjt": pjt, "qb": qb, "kb": kb, "pb": pb,
            "gnw": gn_w.reshape(C, 1), "gnb": gn_b.reshape(C, 1), "gt": gt,
            "on": np.ones((128, 128), np.float32),
        })

    _CACHE["in_maps"] = in_maps
    res = bass_utils.run_bass_kernel_spmd(nc, in_maps, core_ids=list(range(8)))
    y = np.empty((B, C, N), np.float32)
    for core in range(8):
        bb, half = core // 2, core % 2
        y[bb][:, half * NLOC:(half + 1) * NLOC] = res.results[core]["out"]
    return y.reshape(B, C, 64, 64)
